# revision 1
# baseline (speedup 1.0000x reference)
"""Trainium2 Bass kernel for a fake-quantized MLP (qlinear -> gelu -> qlinear).

Reference semantics (B,S,C,H = 32,1024,1024,4096):
    x2d = x.reshape(-1, C)
    h   = round(x2d/sx) @ round(w1/sw1).T * (sx*sw1) + b1 ;  s = max(amax,eps)/127
    g   = gelu(h, exact erf)
    y   = round(g/sh) @ round(w2/sw2).T * (sh*sw2) + b2

Strategy: data-parallel over rows across 8 cores.  Quantized ints fit
exactly in bf16, so matmuls run at full bf16 rate with exact fp32 PSUM
accumulation.  Weight scales need no collective (every core scans the full
weights); the two activation scales (x, gelu output) use tiny
AllReduce(max) collectives.  h (gelu output) is staged to DRAM transposed
as (H, rows) so the second matmul consumes it directly as the stationary
operand.  Scheduling: w1 amax scan gates the front, so the x AllReduce
rides under it and the w2 scan is trickled through phase A; w2
quant+transpose fills the mid-kernel AllReduce bubble.
"""

import sys

import numpy as np

try:
    import concourse.bass as bass
except ImportError:  # pragma: no cover
    sys.path.insert(0, "/opt/trn_rl_repo")
    import concourse.bass as bass

import concourse.mybir as mybir
from contextlib import ExitStack
import concourse.tile as tile
from concourse import masks
from concourse.bass_utils import run_bass_kernel_spmd

from concourse.bass import _add_dep_helper as _add_dep

F32 = mybir.dt.float32
BF16 = mybir.dt.bfloat16
AF = mybir.ActivationFunctionType
ALU = mybir.AluOpType

QP = 127.0
EPS = 1e-6
MAGIC = 12582912.0  # 1.5 * 2**23: fp32 round-to-nearest-even integer trick

# full problem shapes
B, S, C, H = 32, 1024, 1024, 4096
N_CORES = 8


def _split_matmul_waits(nc):
    """This toolchain's walrus codegen allows only ONE sync-wait slot per
    lowered instruction (Matmult waits all land on its LDWEIGHTS since
    --enable-ldw-opt=false; queue DMAs use a single-slot DIRECT2D struct).
    Peel extra waits onto same-engine NoOps inserted just before, except for
    framework-generated drain/barrier instructions which support many."""
    n_split = 0
    for f in nc.m.functions:
        for bb in f.blocks:
            insts = bb.instructions
            out = []
            changed = False
            for inst in insts:
                si = getattr(inst, "sync_info", None)
                if si is not None and si.on_wait and len(si.on_wait) > 1:
                    waits = list(si.on_wait)
                    for k, w in enumerate(waits[:-1]):
                        nop = mybir.InstNoOp(
                            name=f"{inst.name}-wsplit{k}", ins=[], outs=[]
                        )
                        nop.engine = inst.engine
                        nop.sync_info = mybir.SyncInfo(
                            on_wait=[w], on_update=[]
                        )
                        out.append(nop)
                    inst.sync_info = mybir.SyncInfo(
                        on_wait=[waits[-1]], on_update=list(si.on_update or [])
                    )
                    n_split += 1
                    changed = True
                out.append(inst)
            if changed:
                bb.instructions = out
    return n_split


def _dedup_ldweights(nc):
    """Tile legalization emits explicit Ldweights+Matmult pairs, and walrus
    runs with --enable-ldw-opt=false, so every matmul re-streams its
    stationary operand (128 extra PE cycles on a 512-cycle matmul).  Drop an
    Ldweights whose weights AP is identical to the previous one on the PE
    stream (the PE array still holds that stationary); keep its semaphore
    effects on a NoOp."""
    n = 0
    for f in nc.m.functions:
        for bb in f.blocks:
            insts = bb.instructions
            out = []
            last_key = None
            changed = False
            for inst in insts:
                if isinstance(inst, mybir.InstLdweights):
                    key = str(inst.ins[0])
                    if key == last_key:
                        si = getattr(inst, "sync_info", None)
                        if si is not None and (si.on_wait or si.on_update):
                            nop = mybir.InstNoOp(
                                name=inst.name + "-lw", ins=[], outs=[]
                            )
                            nop.engine = inst.engine
                            nop.sync_info = si
                            out.append(nop)
                        n += 1
                        changed = True
                        continue
                    last_key = key
                elif isinstance(inst, mybir.InstMatmult):
                    if inst.is_transpose or getattr(inst, "ldweights", None):
                        last_key = None
                out.append(inst)
            if changed:
                bb.instructions = out
    return n


def build_nc(rows=4096, c=C, h=H, n_cores=N_CORES, gelu="Gelu", split_waits=True):
    """Build the per-core SPMD Bass program.

    rows: rows of x2d handled by each core.
    gelu: "Gelu" (HW ACT table), "Erf" (x*(0.5*erf(x/sqrt2)+0.5)),
          "Identity" (for simulator runs; CoreSim lacks Gelu/Erf).
    """
    assert rows % 512 == 0 and c % 512 == 0 and h % 512 == 0
    nc = bass.Bass()

    x_in = nc.dram_tensor("x", [rows, c], F32, kind="ExternalInput")
    w1_in = nc.dram_tensor("w1", [h, c], F32, kind="ExternalInput")
    b1_in = nc.dram_tensor("b1", [h], F32, kind="ExternalInput")
    w2_in = nc.dram_tensor("w2", [c, h], F32, kind="ExternalInput")
    b2_in = nc.dram_tensor("b2", [c], F32, kind="ExternalInput")
    y_out = nc.dram_tensor("y", [rows, c], F32, kind="ExternalOutput")

    ct = c // 128   # c in 128-blocks
    ht = h // 128   # h in 128-blocks
    n_chunk = rows // 512  # phase B m-chunks
    groups = [list(range(n_cores))]

    with tile.TileContext(nc) as tc, ExitStack() as top:
        consts = top.enter_context(tc.tile_pool(name="consts", bufs=1))
        scal = top.enter_context(tc.tile_pool(name="scal", bufs=1))
        dram = top.enter_context(tc.tile_pool(name="dram", bufs=1, space="DRAM"))

        ident = consts.tile([128, 128], BF16)
        masks.make_identity(nc, ident[:])
        ident_f = consts.tile([128, 128], F32)
        masks.make_identity(nc, ident_f[:])

        # b1 as (128, ht): b1_sb[p, jb] = b1[jb*128 + p]
        b1_sb = consts.tile([128, ht], F32)
        nc.sync.dma_start(
            out=b1_sb[:], in_=b1_in.ap().rearrange("(a b) -> b a", b=128)
        )

        magic_b = consts.tile([128, 1], F32)
        nc.vector.memset(magic_b[:], MAGIC)

        # h scratch in DRAM, transposed: (h, rows)
        h_dram = dram.tile([h, rows], F32)
        # collective bounce buffers (DRAM, non-IO)
        arw_in = dram.tile([2, 1], F32, tag="arwi")
        arw_out = dram.tile([2, 1], F32, tag="arwo")
        arx_in = dram.tile([1, 1], F32, tag="arxi")
        arx_out = dram.tile([1, 1], F32, tag="arxo")
        arh_in = dram.tile([1, 1], F32, tag="arhi")
        arh_out = dram.tile([1, 1], F32, tag="arho")

        # ---------- phase 0: local amaxes ----------
        # Every core scans the full weights (DMA-overlapped with the x scan);
        # amax is order-independent so all cores derive identical scales.
        # w1 scan first: sw1 gates w1 prep which gates the first matmul.
        # Each core scans the FULL weights so the local max is already
        # global -- no collective needed for weight scales.
        wmax1 = scal.tile([128, 1], F32)
        wmax2 = scal.tile([128, 1], F32)
        nc.vector.memset(wmax1[:], 0.0)
        nc.vector.memset(wmax2[:], 0.0)

        def _pscan(src, nrb, ncol, acc, pool, tpool, tag):
            for rb in range(nrb):
                t = pool.tile([128, ncol], F32, tag=tag)
                nc.sync.dma_start(out=t[:], in_=src[rb * 128 : (rb + 1) * 128, :])
                r = tpool.tile([128, 1], F32, tag=tag + "r")
                nc.vector.tensor_reduce(
                    out=r[:], in_=t[:], axis=mybir.AxisListType.X, op=ALU.max,
                    apply_absolute_value=True,
                )
                nc.vector.tensor_tensor(out=acc[:], in0=acc[:], in1=r[:], op=ALU.max)

        def _preduce(acc, psR, tag):
            pt = psR.tile([1, 128], F32, tag=tag)
            nc.tensor.matmul(
                pt[:], lhsT=acc[:], rhs=ident_f[:], start=True, stop=True
            )
            out = scal.tile([1, 1], F32, name=tag + "_r")
            nc.vector.tensor_reduce(
                out=out[:], in_=pt[:], axis=mybir.AxisListType.X, op=ALU.max
            )
            return out

        def _derive(bcast_src_dram, name):
            b = scal.tile([128, 1], F32, name=name + "_b")
            nc.sync.dma_start(out=b[:], in_=bcast_src_dram.to_broadcast((128, 1)))
            s = scal.tile([128, 1], F32, name="s_" + name)
            nc.vector.tensor_scalar(
                out=s[:], in0=b[:], scalar1=EPS, scalar2=float(1.0 / QP),
                op0=ALU.max, op1=ALU.mult,
            )
            inv = scal.tile([128, 1], F32, name="inv_" + name)
            nc.vector.reciprocal(out=inv[:], in_=s[:])
            return s, inv


        # x amax (sharded -> AllReduce max) FIRST: the AllReduce latency
        # hides under the w1 scan that follows.
        xmax = scal.tile([128, 1], F32)
        nc.vector.memset(xmax[:], 0.0)
        with tc.tile_pool(name="x0", bufs=4) as x0p, tc.tile_pool(
            name="x0t", bufs=4
        ) as x0t, tc.tile_pool(name="psRx", bufs=2, space="PSUM") as psR:
            for mb in range(rows // 512):
                t = x0p.tile([128, c * 4], F32, tag="x0a")
                nc.sync.dma_start(
                    out=t[:].rearrange("b (a c) -> b a c", a=4),
                    in_=x_in[mb * 512 : (mb + 1) * 512, :].rearrange(
                        "(a b) c -> b a c", b=128
                    ),
                )
                r = x0t.tile([128, 1], F32, tag="x0r")
                nc.vector.tensor_reduce(
                    out=r[:], in_=t[:], axis=mybir.AxisListType.X, op=ALU.max,
                    apply_absolute_value=True,
                )
                nc.vector.tensor_tensor(
                    out=xmax[:], in0=xmax[:], in1=r[:], op=ALU.max
                )
            xmax_r = _preduce(xmax, psR, "xm")
        nc.gpsimd.dma_start(out=arx_in[:], in_=xmax_r[:])
        nc.gpsimd.collective_compute(
            "AllReduce", ALU.max, replica_groups=groups,
            ins=[arx_in.opt()], outs=[arx_out.opt()],
        )

        # weight scans (w1 first: sw1 gates w1 prep which gates matmul1).
        # Each core scans the FULL weights -> local max is global, no
        # collective needed.
        with tc.tile_pool(name="w0", bufs=6) as w0p, tc.tile_pool(
            name="w0t", bufs=6
        ) as w0t, tc.tile_pool(name="psR", bufs=2, space="PSUM") as psR:
            _pscan(w1_in, ht, c, wmax1, w0p, w0t, "w0a")
            w1max_r = _preduce(wmax1, psR, "w1m")
            nc.gpsimd.dma_start(out=arw_in[0:1, :], in_=w1max_r[:])
            sw1, inv_sw1 = _derive(arw_in[0:1, :], "w1")

        sx, inv_sx = _derive(arx_out, "x")
        sxw1 = scal.tile([128, 1], F32)
        nc.vector.tensor_tensor(out=sxw1[:], in0=sx[:], in1=sw1[:], op=ALU.mult)

        hmax = scal.tile([128, 1], F32)
        nc.vector.memset(hmax[:], 0.0)

        # ---------- w1 quant + transpose -> w1qT[cb] (128, h) bf16 ----------
        w1_stack = ExitStack()
        w1qT_pool = w1_stack.enter_context(tc.tile_pool(name="w1qT", bufs=ct, side="right"))
        w1qT = [w1qT_pool.tile([128, h], BF16, tag="w1qT", name=f"w1qT{i}") for i in range(ct)]

        def quant_transpose(src_dram, n_rowblk, n_col, inv_s, dstT, pools, tagp,
                            on_dve=False):
            """Load (128, 1024)-chunk f32 row-blocks of src, quantize to bf16
            ints, transpose 128x128 blocks via PE, scatter into dstT tiles:
            dstT[global_col_blk][:, rb*128:(rb+1)*128] = q_block.T
            on_dve: run the elementwise passes on DVE (keeps ACT free)."""
            fpool, qpool, pspool = pools
            ck = min(n_col, 1024)
            for rb in range(n_rowblk):
                for jc in range(n_col // ck):
                    t = fpool.tile([128, ck], F32, tag=tagp + "f")
                    nc.sync.dma_start(
                        out=t[:],
                        in_=src_dram[rb * 128 : (rb + 1) * 128,
                                     jc * ck : (jc + 1) * ck],
                    )
                    # pass1: v*inv_s + MAGIC (fp32)
                    if on_dve is True:
                        nc.vector.tensor_scalar(
                            out=t[:], in0=t[:], scalar1=inv_s[:], scalar2=MAGIC,
                            op0=ALU.mult, op1=ALU.add,
                        )
                    else:
                        nc.scalar.activation(
                            out=t[:], in_=t[:], func=AF.Identity, bias=magic_b[:],
                            scale=inv_s[:],
                        )
                    q = qpool.tile([128, ck], BF16, tag=tagp + "q")
                    # pass2: -MAGIC, cast bf16  (DVE)
                    nc.vector.tensor_scalar_add(out=q[:], in0=t[:], scalar1=-MAGIC)
                    for cb in range(ck // 128):
                        ps = pspool.tile([128, 128], F32, tag=tagp + "ps")
                        nc.tensor.matmul(
                            ps[:], lhsT=q[:, cb * 128 : (cb + 1) * 128],
                            rhs=ident[:], start=True, stop=True,
                        )
                        if on_dve:  # True or "split"
                            nc.vector.tensor_copy(
                                out=dstT[jc * (ck // 128) + cb][
                                    :, rb * 128 : (rb + 1) * 128
                                ],
                                in_=ps[:],
                            )
                        else:
                            nc.scalar.copy(
                                out=dstT[jc * (ck // 128) + cb][
                                    :, rb * 128 : (rb + 1) * 128
                                ],
                                in_=ps[:],
                            )

        with tc.tile_pool(name="wqf", bufs=3) as fp, tc.tile_pool(
            name="wqq", bufs=3
        ) as qp, tc.tile_pool(name="wqps", bufs=4, space="PSUM") as pp:
            quant_transpose(w1_in, ht, c, inv_sw1, w1qT, (fp, qp, pp), "w1")


        # ---------- phase A: h.T = gelu(w1q @ xq.T * (sx*sw1) + b1) ----------
        # chunk = 1024 rows: per (jb, cb) the stationary w1qT block feeds two
        # 512-wide matmuls (ms), letting the ldweights post-pass drop half the
        # stationary loads.
        CH = min(1024, rows)
        n_ms = CH // 512
        with tc.tile_pool(name="w02", bufs=2) as w02p, tc.tile_pool(
            name="w02t", bufs=2
        ) as w02t, tc.tile_pool(name="xa", bufs=6) as xa, tc.tile_pool(
            name="xqt", bufs=2 * ct
        ) as xqtp, tc.tile_pool(name="gs", bufs=10) as gs, tc.tile_pool(
            name="gr", bufs=10
        ) as gr, tc.tile_pool(name="psT", bufs=2, space="PSUM") as psT, tc.tile_pool(
            name="psH", bufs=3 * n_ms, space="PSUM"
        ) as psH:
            n_chunks_a = rows // CH
            w2_per_chunk = (ct + n_chunks_a - 1) // n_chunks_a
            for mc in range(rows // CH):
                # trickle the w2 amax scan (2 row-blocks per chunk): its DMA
                # rides under phase A without starving the x reloads
                for k in range(w2_per_chunk):
                    ob = mc * w2_per_chunk + k
                    if ob < ct:
                        wt = w02p.tile([128, h], F32, tag="w0b")
                        nc.sync.dma_start(
                            out=wt[:], in_=w2_in[ob * 128 : (ob + 1) * 128, :]
                        )
                        wr = w02t.tile([128, 1], F32, tag="w0br")
                        nc.vector.tensor_reduce(
                            out=wr[:], in_=wt[:], axis=mybir.AxisListType.X,
                            op=ALU.max, apply_absolute_value=True,
                        )
                        nc.vector.tensor_tensor(
                            out=wmax2[:], in0=wmax2[:], in1=wr[:], op=ALU.max
                        )
                # quantize x row-tiles, transpose into xqT[cb] (128c, CH m)
                xqT = [xqtp.tile([128, CH], BF16, tag="xqT", name=f"xqT{mc}_{i}") for i in range(ct)]
                for t8 in range(CH // 128):
                    m0 = mc * CH + t8 * 128
                    xt = xa.tile([128, c], F32, tag="xa")
                    nc.sync.dma_start(out=xt[:], in_=x_in[m0 : m0 + 128, :])
                    nc.scalar.activation(
                        out=xt[:], in_=xt[:], func=AF.Identity, bias=magic_b[:],
                        scale=inv_sx[:],
                    )
                    xq = xa.tile([128, c], BF16, tag="xq")
                    nc.vector.tensor_scalar_add(out=xq[:], in0=xt[:], scalar1=-MAGIC)
                    for cq in range(ct // 4):
                        ps = psT.tile([128, 512], F32, tag="psT")
                        for c4 in range(4):
                            cb = cq * 4 + c4
                            nc.tensor.matmul(
                                ps[:, c4 * 128 : (c4 + 1) * 128],
                                lhsT=xq[:, cb * 128 : (cb + 1) * 128],
                                rhs=ident[:], start=True, stop=True,
                            )
                        for c4 in range(4):
                            cb = cq * 4 + c4
                            nc.scalar.copy(
                                out=xqT[cb][:, t8 * 128 : (t8 + 1) * 128],
                                in_=ps[:, c4 * 128 : (c4 + 1) * 128],
                            )
                # matmul over j blocks; gelu; amax; store h.T
                for jb in range(ht):
                    phs = [
                        psH.tile([128, 512], F32, tag="psH", name=f"psH{mc}_{jb}_{i}")
                        for i in range(n_ms)
                    ]
                    prev = None
                    for cb in range(ct):
                        for ms in range(n_ms):
                            mmi = nc.tensor.matmul(
                                phs[ms][:],
                                lhsT=w1qT[cb][:, jb * 128 : (jb + 1) * 128],
                                rhs=xqT[cb][:, ms * 512 : (ms + 1) * 512],
                                start=(cb == 0),
                                stop=(cb == ct - 1),
                            )
                            if prev is not None:
                                _add_dep(mmi.ins, prev.ins, sync=False,
                                         reason="ldw-order")
                            prev = mmi
                    for ms in range(n_ms):
                        ph = phs[ms]
                        g = gs.tile([128, 512], F32, tag="gs")
                        if gelu == "Erf":
                            # g = h*(0.5*erf(h/sqrt2)+0.5);  h = ph*sxw1 + b1
                            hh = gs.tile([128, 512], F32, tag="gh")
                            nc.scalar.activation(
                                out=hh[:], in_=ph[:], func=AF.Identity,
                                bias=b1_sb[:, jb : jb + 1], scale=sxw1[:],
                            )
                            e = gs.tile([128, 512], F32, tag="ge")
                            nc.scalar.activation(
                                out=e[:], in_=hh[:], func=AF.Erf, bias=0.0,
                                scale=float(1.0 / np.sqrt(2.0)),
                            )
                            nc.vector.tensor_scalar(
                                out=e[:], in0=e[:], scalar1=0.5, scalar2=0.5,
                                op0=ALU.mult, op1=ALU.add,
                            )
                            nc.vector.tensor_tensor(
                                out=g[:], in0=e[:], in1=hh[:], op=ALU.mult
                            )
                        else:
                            nc.scalar.activation(
                                out=g[:], in_=ph[:], func=getattr(AF, gelu),
                                bias=b1_sb[:, jb : jb + 1], scale=sxw1[:],
                            )
                        r = gr.tile([128, 1], F32, tag="gr")
                        nc.vector.tensor_reduce(
                            out=r[:], in_=g[:], axis=mybir.AxisListType.X,
                            op=ALU.max, apply_absolute_value=True,
                        )
                        nc.vector.tensor_tensor(
                            out=hmax[:], in0=hmax[:], in1=r[:], op=ALU.max
                        )
                        m0 = mc * CH + ms * 512
                        nc.sync.dma_start(
                            out=h_dram[jb * 128 : (jb + 1) * 128, m0 : m0 + 512],
                            in_=g[:],
                        )

        w1_stack.close()

        with tc.tile_pool(name="psR2", bufs=2, space="PSUM") as psR:
            w2max_r = _preduce(wmax2, psR, "w2m")
        nc.gpsimd.dma_start(out=arw_in[1:2, :], in_=w2max_r[:])

        # ---------- w2 quant + transpose -> w2qT[jb] (128, c) bf16 ----------
        # emitted after phase A so its DMA/ACT/PE work fills the AR_h bubble
        sw2, inv_sw2 = _derive(arw_in[1:2, :], "w2")
        w2_stack = ExitStack()
        w2qT_pool = w2_stack.enter_context(tc.tile_pool(name="w2qT", bufs=ht))
        w2qT = [w2qT_pool.tile([128, c], BF16, tag="w2qT", name=f"w2qT{i}") for i in range(ht)]
        with tc.tile_pool(name="w2f", bufs=3) as fp, tc.tile_pool(
            name="w2q", bufs=3
        ) as qp, tc.tile_pool(name="w2ps", bufs=4, space="PSUM") as pp:
            quant_transpose(w2_in, ct, h, inv_sw2, w2qT, (fp, qp, pp), "w2",
                            on_dve="split")

        # ---------- h scale: AllReduce ----------
        with tc.tile_pool(name="psRh", bufs=2, space="PSUM") as psR:
            hmax_t = psR.tile([1, 128], F32, tag="psRh")
            nc.tensor.matmul(
                hmax_t[:], lhsT=hmax[:], rhs=ident_f[:], start=True, stop=True
            )
            hmax_r = scal.tile([1, 1], F32)
            nc.vector.tensor_reduce(
                out=hmax_r[:], in_=hmax_t[:], axis=mybir.AxisListType.X,
                op=ALU.max,
            )
        nc.gpsimd.dma_start(out=arh_in[:], in_=hmax_r[:])
        nc.gpsimd.collective_compute(
            "AllReduce", ALU.max, replica_groups=groups,
            ins=[arh_in.opt()], outs=[arh_out.opt()],
        )
        gh_b = scal.tile([128, 1], F32)
        nc.sync.dma_start(out=gh_b[:], in_=arh_out.to_broadcast((128, 1)))
        sh = scal.tile([128, 1], F32)
        nc.vector.tensor_scalar(
            out=sh[:], in0=gh_b[:], scalar1=EPS, scalar2=float(1.0 / QP),
            op0=ALU.max, op1=ALU.mult,
        )
        inv_sh = scal.tile([128, 1], F32)
        nc.vector.reciprocal(out=inv_sh[:], in_=sh[:])
        shw2 = scal.tile([128, 1], F32)
        nc.vector.tensor_tensor(out=shw2[:], in0=sh[:], in1=sw2[:], op=ALU.mult)

        # ---------- phase B: y = hq.T.T @ w2q.T * (sh*sw2) + b2 ----------
        with tc.tile_pool(name="b2p", bufs=1) as b2p, tc.tile_pool(
            name="hb", bufs=8
        ) as hb, tc.tile_pool(name="hqt", bufs=3) as hqtp, tc.tile_pool(
            name="ys", bufs=4
        ) as ys, tc.tile_pool(name="psY", bufs=3 * (c // 512), space="PSUM") as psY:
            b2_b = b2p.tile([128, c], F32)
            nc.sync.dma_start(
                out=b2_b[:],
                in_=b2_in.ap().rearrange("(o a) -> o a", o=1).to_broadcast((128, c)),
            )

            for mc in range(n_chunk):
                hqT = hqtp.tile([128, ht * 512], BF16, tag="hqT")
                for jb in range(ht):
                    th = hb.tile([128, 512], F32, tag="hb")
                    nc.sync.dma_start(
                        out=th[:],
                        in_=h_dram[jb * 128 : (jb + 1) * 128,
                                   mc * 512 : (mc + 1) * 512],
                    )
                    nc.scalar.activation(
                        out=th[:], in_=th[:], func=AF.Identity, bias=magic_b[:],
                        scale=inv_sh[:],
                    )
                    nc.vector.tensor_scalar_add(
                        out=hqT[:, jb * 512 : (jb + 1) * 512], in0=th[:],
                        scalar1=-MAGIC,
                    )
                n_ob = c // 512
                for ms in range(4):
                    pys = [
                        psY.tile([128, 512], F32, tag="psY", name=f"psY{mc}_{ms}_{i}")
                        for i in range(n_ob)
                    ]
                    prev = None
                    for jb in range(ht):
                        for ob in range(n_ob):
                            mmi = nc.tensor.matmul(
                                pys[ob][:],
                                lhsT=hqT[:, jb * 512 + ms * 128 :
                                         jb * 512 + (ms + 1) * 128],
                                rhs=w2qT[jb][:, ob * 512 : (ob + 1) * 512],
                                start=(jb == 0),
                                stop=(jb == ht - 1),
                            )
                            if prev is not None:
                                _add_dep(mmi.ins, prev.ins, sync=False,
                                         reason="ldw-order")
                            prev = mmi
                    for ob in range(n_ob):
                        yt = ys.tile([128, 512], F32, tag="ys")
                        nc.vector.scalar_tensor_tensor(
                            out=yt[:], in0=pys[ob][:], scalar=shw2[:],
                            in1=b2_b[:, ob * 512 : (ob + 1) * 512],
                            op0=ALU.mult, op1=ALU.add,
                        )
                        m0 = mc * 512 + ms * 128
                        nc.sync.dma_start(
                            out=y_out[m0 : m0 + 128, ob * 512 : (ob + 1) * 512],
                            in_=yt[:],
                        )

        w2_stack.close()

    if split_waits:
        _split_matmul_waits(nc)
        _dedup_ldweights(nc)
    return nc


_CACHED = {}


def _get_nc(rows, c, h, n_cores, gelu):
    key = (rows, c, h, n_cores, gelu)
    if key not in _CACHED:
        _CACHED[key] = build_nc(rows=rows, c=c, h=h, n_cores=n_cores, gelu=gelu)
    return _CACHED[key]


def run(inputs, trace=False, gelu="Gelu", n_cores=N_CORES):
    x = np.asarray(inputs["x"], np.float32)
    w1 = np.ascontiguousarray(np.asarray(inputs["w1"], np.float32))
    b1 = np.ascontiguousarray(np.asarray(inputs["b1"], np.float32))
    w2 = np.ascontiguousarray(np.asarray(inputs["w2"], np.float32))
    b2 = np.ascontiguousarray(np.asarray(inputs["b2"], np.float32))
    b_, s_, c_ = x.shape
    h_ = w1.shape[0]
    x2d = np.ascontiguousarray(x.reshape(-1, c_))
    rows = x2d.shape[0] // n_cores
    nc = _get_nc(rows, c_, h_, n_cores, gelu)
    in_maps = [
        {
            "x": np.ascontiguousarray(x2d[i * rows : (i + 1) * rows]),
            "w1": w1,
            "b1": b1,
            "w2": w2,
            "b2": b2,
        }
        for i in range(n_cores)
    ]
    res = run_bass_kernel_spmd(nc, in_maps, list(range(n_cores)), trace=trace)
    y2d = np.concatenate([r["y"] for r in res.results], axis=0)
    return y2d.reshape(b_, s_, c_).astype(np.float32), res


def kernel(x, w1, b1, w2, b2):
    y, _ = run({"x": x, "w1": w1, "b1": b1, "w2": w2, "b2": b2})
    return y

